# revision 7
# baseline (speedup 1.0000x reference)
"""LinearRNNBlock Trainium2 kernel.

B=8, T=2048, C=1024, EXP=4. Data-parallel over batch: core b computes batch b.

On-chip layout is feature-major [c partitions, t free] end to end, bf16
activations/weights with fp32 PSUM accumulation and an fp32 scan state:
  - host pre-transposes x[b] -> [NB, P, CK, TB] bf16 t-blocks and pre-tiles
    all weights into [K=128, M=128] bf16 lhsT blocks (zero device transposes)
  - rmsnorm reductions (over C = partitions) go through a ones-matmul on the
    PE whose M=128 output is the per-token sum broadcast across partitions;
    rsqrt is exp(-0.5*ln(ms+eps)) on ScalarE
  - cumsum over T is a DVE tensor_tensor_scan along the free dim (fp32
    internal state even with bf16 operands), chained across t-blocks
  - the program is emitted software-pipelined: the norm1/scan/mlp1/norm2
    phase of t-blocks 2k+2,2k+3 is interleaved into the mlp2 phase of
    super-block k so the PE never waits on the DVE scan chain
  - mlp2 weights are streamed once per 1024-token super-block (two t-blocks
    share each weight load), halving weight DMA traffic
"""

import os
import sys

sys.path.insert(0, "/opt/trn_rl_repo")

from contextlib import ExitStack

import ml_dtypes
import numpy as np

import concourse.bass as bass
import concourse.tile as tile
from concourse import bacc
from concourse import mybir
from concourse.bass import ts
from concourse.bass_utils import run_bass_kernel_spmd

P = 128
B = 8
T = 2048
C = 1024
E = 4096
CK = C // P   # 8 channel chunks
EK = E // P   # 32 expanded chunks
TB = 512      # t-block (one PSUM bank of fp32)
NB = T // TB  # 4 t-blocks
NSB = NB // 2  # 2 super-blocks (pairs of t-blocks sharing mlp2 weight loads)
EPS = 1e-6

F32 = mybir.dt.float32
BF16 = mybir.dt.bfloat16
AF = mybir.ActivationFunctionType
OP = mybir.AluOpType
BF = ml_dtypes.bfloat16

N_CORES = 8

_CACHED = {}


def _build_program():
    nc = bacc.Bacc("TRN2", target_bir_lowering=False, debug=False,
                   enable_asserts=False, num_devices=N_CORES)

    xt = nc.dram_tensor("xt", [NB, P, CK, TB], BF16, kind="ExternalInput").ap()
    w1 = nc.dram_tensor("w1", [CK, P, CK * P], BF16, kind="ExternalInput").ap()
    b1 = nc.dram_tensor("b1", [P, CK], F32, kind="ExternalInput").ap()
    w21 = nc.dram_tensor("w21", [EK, P, CK * P], BF16, kind="ExternalInput").ap()
    b21 = nc.dram_tensor("b21", [P, EK], F32, kind="ExternalInput").ap()
    w22 = nc.dram_tensor("w22", [CK, 4, P, 8 * P], BF16, kind="ExternalInput").ap()
    b22 = nc.dram_tensor("b22", [P, CK], F32, kind="ExternalInput").ap()
    sci = nc.dram_tensor("sci", [P, T], BF16, kind="ExternalInput").ap()
    onesd = nc.dram_tensor("onesd", [P, P], BF16, kind="ExternalInput").ap()
    yt = nc.dram_tensor("yt", [CK, P, T], F32, kind="ExternalOutput").ap()

    with tile.TileContext(nc) as tc, ExitStack() as ctx:
        consts = ctx.enter_context(tc.tile_pool(name="consts", bufs=1))
        arena = ctx.enter_context(tc.tile_pool(name="arena", bufs=1))
        wp = ctx.enter_context(tc.tile_pool(name="wp", bufs=6))
        wq = ctx.enter_context(tc.tile_pool(name="wq", bufs=6))
        sp = ctx.enter_context(tc.tile_pool(name="sp", bufs=2))
        h2p = ctx.enter_context(tc.tile_pool(name="h2p", bufs=2))
        apool = ctx.enter_context(tc.tile_pool(name="apool", bufs=1))
        rp = ctx.enter_context(tc.tile_pool(name="rp", bufs=2))
        gp = ctx.enter_context(tc.tile_pool(name="gp", bufs=4))
        sqp = ctx.enter_context(tc.tile_pool(name="sqp", bufs=4))
        yp = ctx.enter_context(tc.tile_pool(name="yp", bufs=4))
        ps = ctx.enter_context(tc.tile_pool(name="ps", bufs=1, space="PSUM"))

        # x t-block 0 and the small constants first so the prologue starts
        # immediately; the 2 MB w1 load follows (needed only ~12 us in)
        xa = arena.tile([P, NB, CK, TB], BF16)
        for cc in range(CK):
            nc.sync.dma_start(out=xa[:, 0, cc], in_=xt[0][:, cc])

        ones = consts.tile([P, P], BF16)
        nc.sync.dma_start(out=ones, in_=onesd)
        epsb = consts.tile([P, 1], F32)
        nc.vector.memset(epsb, EPS)
        b1s = consts.tile([P, CK], F32)
        nc.sync.dma_start(out=b1s, in_=b1)
        b21s = consts.tile([P, EK], F32)
        nc.sync.dma_start(out=b21s, in_=b21)
        b22s = consts.tile([P, CK], F32)
        nc.sync.dma_start(out=b22s, in_=b22)
        scib = consts.tile([P, T], BF16)
        nc.sync.dma_start(out=scib, in_=sci)

        w1sb = consts.tile([P, CK, CK, P], BF16)
        for dc in range(CK):
            nc.sync.dma_start(out=w1sb[:, dc], in_=w1[dc])

        # per-tb state / h2 (bf16); per-SB relu activations
        carries = [None] * CK

        def emit_norm1(tb):
            if tb > 0:
                nc.sync.dma_start(out=xa[:, tb], in_=xt[tb])
            acc = ps.tile([P, TB], F32, tag="acc", bufs=2, name=f"acc1_{tb}")
            for cc in range(CK):
                sq = sqp.tile([P, TB], BF16, tag="sq", name="sq1")
                nc.vector.tensor_mul(sq, xa[:, tb, cc], xa[:, tb, cc])
                nc.tensor.matmul(acc, lhsT=ones, rhs=sq,
                                 start=(cc == 0), stop=(cc == CK - 1))
            ln1 = gp.tile([P, TB], F32, tag="ln", bufs=2, name="ln1")
            nc.scalar.activation(ln1, acc, AF.Ln, bias=epsb, scale=1.0 / C)
            r1 = rp.tile([P, TB], BF16, tag="rstd1", bufs=2, name="rstd1")
            nc.scalar.activation(r1, ln1, AF.Exp, scale=-0.5)
            return r1

        def emit_mlp1_scan(tb, r1):
            # h1 = x*rstd1, then v = h1 @ W1 on the PE immediately (no scan
            # wait: cumsum(h1)@W1 == cumsum(h1@W1)). The scan runs per output
            # chunk on the PSUM result, then gate=sigmoid(scan*sci + b1) and
            # out1 = gate*x in place.
            h1t = sp.tile([P, CK, TB], BF16, tag="h1", bufs=2, name="h1t")
            for cc in range(CK):
                nc.vector.tensor_mul(h1t[:, cc], xa[:, tb, cc], r1)
            for dc in range(CK):
                pg = ps.tile([P, TB], F32, tag="mm", bufs=4, name="pg")
                for cc in range(CK):
                    nc.tensor.matmul(pg, lhsT=w1sb[:, dc, cc], rhs=h1t[:, cc],
                                     start=(cc == 0), stop=(cc == CK - 1))
                raw = gp.tile([P, TB], BF16, tag="raw", bufs=3, name="raw")
                init = 0.0 if tb == 0 else carries[dc]
                nc.vector.tensor_tensor_scan(raw, pg, scib[:, ts(tb, TB)],
                                             initial=init,
                                             op0=OP.add, op1=OP.bypass)
                carry = gp.tile([P, 1], BF16, tag="carry", bufs=CK + 2,
                                name="carry")
                nc.vector.tensor_copy(carry, raw[:, TB - 1:TB])
                carries[dc] = carry
                logit = gp.tile([P, TB], BF16, tag="logit", bufs=3, name="lg")
                nc.vector.tensor_mul(logit, raw, scib[:, ts(tb, TB)])
                g = gp.tile([P, TB], BF16, tag="g", name="g")
                nc.scalar.activation(g, logit, AF.Sigmoid,
                                     bias=b1s[:, dc:dc + 1], scale=1.0)
                nc.vector.tensor_mul(xa[:, tb, dc], g, xa[:, tb, dc])

        def emit_norm2(tb):
            # norm2 on out1 (in xa)
            acc2 = ps.tile([P, TB], F32, tag="acc", bufs=2, name=f"acc2_{tb}")
            for cc in range(CK):
                sq2 = sqp.tile([P, TB], BF16, tag="sq", name="sq2")
                nc.vector.tensor_mul(sq2, xa[:, tb, cc], xa[:, tb, cc])
                nc.tensor.matmul(acc2, lhsT=ones, rhs=sq2,
                                 start=(cc == 0), stop=(cc == CK - 1))
            ln2 = gp.tile([P, TB], F32, tag="ln", bufs=2, name="ln2")
            nc.scalar.activation(ln2, acc2, AF.Ln, bias=epsb, scale=1.0 / C)
            r2 = rp.tile([P, TB], BF16, tag="rstd2", bufs=2, name="rstd2")
            nc.scalar.activation(r2, ln2, AF.Exp, scale=-0.5)
            h2t = h2p.tile([P, CK, TB], BF16, tag="h2", bufs=2, name="h2t")
            for cc in range(CK):
                nc.vector.tensor_mul(h2t[:, cc], xa[:, tb, cc], r2)
            return h2t

        def emit_mlp2a(h2ts, at):
            # a = relu(h2 @ W21 + b21), both t-blocks per weight load
            for ec in range(EK):
                w21s = wp.tile([P, CK, P], BF16, tag="w21", name="w21s")
                nc.sync.dma_start(out=w21s, in_=w21[ec])
                for h in range(2):
                    pa = ps.tile([P, TB], F32, tag="mm", bufs=4, name="pa")
                    for cc in range(CK):
                        nc.tensor.matmul(pa, lhsT=w21s[:, cc], rhs=h2ts[h][:, cc],
                                         start=(cc == 0), stop=(cc == CK - 1))
                    nc.scalar.activation(at[:, h, ec], pa, AF.Relu,
                                         bias=b21s[:, ec:ec + 1], scale=1.0)

        def emit_mlp2b(k, at):
            # y = a @ W22 + b22 + out1, both t-blocks per weight load.
            # (dc,h) groups run back-to-back so each py drain overlaps the
            # other half's 32-MM accumulation instead of stalling the PE.
            def load_w22(dc):
                l = []
                for q in range(4):
                    w22s = wq.tile([P, 8, P], BF16, tag="w22", name="w22s")
                    nc.sync.dma_start(out=w22s, in_=w22[dc, q])
                    l.append(w22s)
                return l

            w22_next = load_w22(0)
            for dc in range(CK):
                w22l = w22_next
                if dc + 1 < CK:
                    w22_next = load_w22(dc + 1)
                for h in range(2):
                    py = ps.tile([P, TB], F32, tag="py", bufs=2, name="py")
                    for q in range(4):
                        for j in range(8):
                            nc.tensor.matmul(py, lhsT=w22l[q][:, j],
                                             rhs=at[:, h, q * 8 + j],
                                             start=(q == 0 and j == 0),
                                             stop=(q == 3 and j == 7))
                    tb = 2 * k + h
                    y = yp.tile([P, TB], F32, tag="y", name="y")
                    nc.vector.scalar_tensor_tensor(out=y, in0=py,
                                                   scalar=b22s[:, dc:dc + 1],
                                                   in1=xa[:, tb, dc],
                                                   op0=OP.add, op1=OP.add)
                    nc.sync.dma_start(out=yt[dc][:, ts(tb, TB)], in_=y)

        # ---- software-pipelined emission ----
        r1_0 = emit_norm1(0)
        emit_mlp1_scan(0, r1_0)
        r1_1 = emit_norm1(1)
        emit_mlp1_scan(1, r1_1)
        h2s = {0: emit_norm2(0), 1: emit_norm2(1)}
        for k in range(NSB):
            at = apool.tile([P, 2, EK, TB], BF16, tag="a", bufs=1, name="at")
            if k + 1 < NSB:
                r1_2 = emit_norm1(2 * k + 2)
            emit_mlp2a([h2s[2 * k], h2s[2 * k + 1]], at)
            if k + 1 < NSB:
                emit_mlp1_scan(2 * k + 2, r1_2)
                r1_3 = emit_norm1(2 * k + 3)
                emit_mlp1_scan(2 * k + 3, r1_3)
                h2s[2 * k + 2] = emit_norm2(2 * k + 2)
                h2s[2 * k + 3] = emit_norm2(2 * k + 3)
            emit_mlp2b(k, at)

    nc.compile()
    return nc


def _prep_weights(norm1_w, mlp1_w, mlp1_b, norm2_w, mlp2_w1, mlp2_b1, mlp2_w2,
                  mlp2_b2):
    W1 = (np.asarray(norm1_w, np.float32)[:, None]
          * np.asarray(mlp1_w, np.float32))
    W21 = (np.asarray(norm2_w, np.float32)[:, None]
           * np.asarray(mlp2_w1, np.float32))
    W22 = np.asarray(mlp2_w2, np.float32)

    w1t = np.ascontiguousarray(
        W1.reshape(CK, P, CK, P).transpose(2, 1, 0, 3).reshape(CK, P, CK * P)
    ).astype(BF)
    w21t = np.ascontiguousarray(
        W21.reshape(CK, P, EK, P).transpose(2, 1, 0, 3).reshape(EK, P, CK * P)
    ).astype(BF)
    w22t = np.ascontiguousarray(
        W22.reshape(4, 8, P, CK, P).transpose(3, 0, 2, 1, 4)
        .reshape(CK, 4, P, 8 * P)).astype(BF)

    b1t = np.ascontiguousarray(np.asarray(mlp1_b, np.float32).reshape(CK, P).T)
    b21t = np.ascontiguousarray(np.asarray(mlp2_b1, np.float32).reshape(EK, P).T)
    b22t = np.ascontiguousarray(np.asarray(mlp2_b2, np.float32).reshape(CK, P).T)

    scaler = np.cumsum(np.arange(1, T + 1, dtype=np.float64))
    sci_b = np.ascontiguousarray(
        np.broadcast_to((1.0 / scaler).astype(BF), (P, T)))

    return dict(w1=w1t, b1=b1t, w21=w21t, b21=b21t, w22=w22t, b22=b22t,
                sci=sci_b, onesd=np.ones((P, P), BF))


LAST_RESULTS = None


def kernel(x, norm1_w, mlp1_w, mlp1_b, norm2_w, mlp2_w1, mlp2_b1, mlp2_w2,
           mlp2_b2):
    global LAST_RESULTS
    x = np.asarray(x, np.float32)
    assert x.shape == (B, T, C), x.shape

    if "nc" not in _CACHED:
        _CACHED["nc"] = _build_program()
    nc = _CACHED["nc"]

    weights = _prep_weights(norm1_w, mlp1_w, mlp1_b, norm2_w,
                            mlp2_w1, mlp2_b1, mlp2_w2, mlp2_b2)

    in_maps = []
    for b in range(B):
        # [T, C] -> [NB, P(c within chunk), CK, TB]
        xt_b = np.ascontiguousarray(
            x[b].T.reshape(CK, P, NB, TB).transpose(2, 1, 0, 3)).astype(BF)
        in_maps.append(dict(xt=xt_b, **weights))

    trace = bool(int(os.environ.get("KERNEL_TRACE", "0")))
    res = run_bass_kernel_spmd(nc, in_maps, core_ids=list(range(N_CORES)),
                               trace=trace)
    LAST_RESULTS = res

    y = np.stack([r["yt"].reshape(C, T).T for r in res.results])
    return np.ascontiguousarray(y.astype(np.float32))
